# revision 1
# baseline (speedup 1.0000x reference)
"""Int8 quantized linear (x @ W^T with per-token requant) on 8 TRN2 NeuronCores.

Strategy: column-parallel over out_features N=11008 -> 1376 per core.
 - x (2048,4096) int8 is replicated; W shard (1376,4096) int8 per core.
 - int8 values are converted to bf16 on host (exact: |v| <= 127 < 256) and
   pre-transposed so the contraction dim K lands on SBUF partitions.
 - PE does bf16 matmuls accumulating exactly into fp32 PSUM (|acc| << 2^24).
 - ScalarE fuses requant: out_int8 = sat_rne(acc_f32 * requant_scale[token]),
   which bit-matches clip(round(acc * rs), -128, 127) of the reference.
"""
import sys

if '/opt/trn_rl_repo' not in sys.path:
    sys.path.insert(0, '/opt/trn_rl_repo')

import numpy as np
import ml_dtypes

import concourse.bass as bass
import concourse.mybir as mybir
import concourse.tile as tile
from concourse import bacc
from concourse.bass_utils import run_bass_kernel_spmd

T, K, N = 2048, 4096, 11008
NCORES = 8
NS = N // NCORES            # 1376 out-features per core
P = 128
TT = T // P                 # 16 token tiles
KT = K // P                 # 32 contraction tiles
N_CHUNKS = [(0, 512), (512, 512), (1024, 352)]   # bank-aligned psum chunks

_cached = None


def _build_program():
    nc = bacc.Bacc("TRN2", target_bir_lowering=False, debug=False)

    xt_d = nc.dram_tensor("xt", [P, TT, KT, P], mybir.dt.bfloat16,
                          kind="ExternalInput")
    wt_d = nc.dram_tensor("wt", [P, KT, NS], mybir.dt.bfloat16,
                          kind="ExternalInput")
    rs_d = nc.dram_tensor("rs", [TT, P, 1], mybir.dt.float32,
                          kind="ExternalInput")
    out_d = nc.dram_tensor("out", [TT, P, NS], mybir.dt.int8,
                           kind="ExternalOutput")

    with tile.TileContext(nc) as tc:
        with (
            tc.tile_pool(name="w", bufs=1) as wp,
            tc.tile_pool(name="x", bufs=3) as xp,
            tc.tile_pool(name="r", bufs=3) as rp,
            tc.tile_pool(name="o", bufs=3) as op,
            tc.tile_pool(name="ps", bufs=2, space="PSUM") as pp,
        ):
            wt = wp.tile([P, KT, NS], mybir.dt.bfloat16)
            nc.sync.dma_start(wt[:], wt_d[:])

            for i in range(TT):
                xt = xp.tile([P, KT, P], mybir.dt.bfloat16, tag="xt")
                nc.sync.dma_start(xt[:], xt_d[:, i])
                rs = rp.tile([P, 1], mybir.dt.float32, tag="rs")
                nc.sync.dma_start(rs[:], rs_d[i])

                acc = pp.tile([P, NS], mybir.dt.float32, tag="acc")
                for k in range(KT):
                    for (n0, nw) in N_CHUNKS:
                        nc.tensor.matmul(
                            acc[:, n0:n0 + nw],
                            xt[:, k, :],
                            wt[:, k, n0:n0 + nw],
                            start=(k == 0),
                            stop=(k == KT - 1),
                        )

                ot = op.tile([P, NS], mybir.dt.int8, tag="ot")
                nc.scalar.activation(ot[:], acc[:],
                                     mybir.ActivationFunctionType.Copy,
                                     scale=rs[:, 0:1])
                nc.sync.dma_start(out_d[i], ot[:])

    nc.compile()
    return nc


def kernel(x, weight_q, scale_x, scale_w, scale_y):
    global _cached
    if _cached is None:
        _cached = _build_program()
    nc = _cached

    # host-side prep (exact)
    rs = (scale_x * scale_w / scale_y).astype(np.float32)          # (T,)
    rs_t = np.ascontiguousarray(rs.reshape(TT, P, 1))

    # xt[p, i, kt, tt] = x[i*128+tt, kt*128+p] : K on partitions, tiled
    xt = np.ascontiguousarray(
        x.reshape(TT, P, KT, P).transpose(3, 0, 2, 1)
    ).astype(ml_dtypes.bfloat16)

    in_maps = []
    for c in range(NCORES):
        wc = weight_q[c * NS:(c + 1) * NS, :]                      # (NS, K)
        # wt[p, kt, n] = wc[n, kt*128+p]
        wt = np.ascontiguousarray(
            wc.reshape(NS, KT, P).transpose(2, 1, 0)
        ).astype(ml_dtypes.bfloat16)
        in_maps.append({"xt": xt, "wt": wt, "rs": rs_t})

    res = run_bass_kernel_spmd(nc, in_maps, list(range(NCORES))).results

    out = np.concatenate(
        [res[c]["out"].reshape(T, NS) for c in range(NCORES)], axis=1
    ).astype(np.int8)
    return out, np.asarray(scale_y, dtype=np.float32)
